# revision 56
# baseline (speedup 1.0000x reference)
"""Trainium2 Bass kernel for MeshInterpolate (interpolate_face_attributes).

Problem (hardcoded shapes):
  pix_to_face [4, 512, 512, 1] int64 (-1 = background), values in [-1, 10000)
  bary_coords [4, 512, 512, 1, 3] f32
  face_memory [10000, 3, 128] f32
  output      [4, 128, 512, 512] f32 (NCHW)

Sharding: data-parallel over (N, H/2): 8 cores, core c handles image c//2,
rows 256*(c%2) .. +256  -> 131072 pixels per core. face_memory replicated.

v2 design (all fp16 on device; host casts the output back to f32):
  - face_memory rows pre-interleaved on host to [f, h(64), v(3), l(2)] fp16
    (channel c = 2h+l) so the bary product runs in DVE 2x mode with a
    pair-duplicated bary operand (inner dim stride-1 size-2 on every AP).
  - dma_gather(prepare_only=True) + trigger_dma: Pool only pays descriptor
    generation (~2us/1024 rows); SDMA drains overlap the next prep.
    Background pixels index a zero row appended at fm[10000] -> no masking.
  - per tile (T = 2048 pixels = G=16 blocks of 128):
      prod[p,g,h,v,l] = attrs[p,g,h,v,l] * bary2[p,g,-,v,l]   (1 DVE op, 2x)
      s01 = prod[v=0] + prod[v=1]; sum = s01 + prod[v=2]      (2 DVE ops, 2x)
      PE is_transpose matmul per 128-px block: psum[c, p'] (fp16 1 cyc/row)
      ACT copies psum f32 -> fp16 bounce; HWDGE DMA to out[128, npix] fp16.
"""

import os

import numpy as np

# Safety: recover wedged NeuronCores from a previous crashed process. Must be
# set before the first jax/NRT backend init in this process.
os.environ.setdefault("NEURON_RT_RESET_CORES", "1")

P = 128
ELEM = 384            # one face row: 3*128 fp16 = 768B
G = 16                # 128-pixel blocks per tile
T = G * P             # 2048 pixels per tile
CHUNK = 1024          # pixels per dma_gather call; each chunk gathers into
                      # its OWN tile (Tile misses consumer waits when two
                      # preps write disjoint slices of one tile). 2048 hangs
                      # the gather ucode on HW - do not raise.
NCHUNK = T // CHUNK   # gather chunks per tile
GPC = CHUNK // P      # g-blocks per chunk
NTILES = 64           # per-core tiles: 131072 pixels
F = 10000
N_CORES = 8
NPIX_CORE = NTILES * T

_CACHE = {}


def _build_nc(ntiles=NTILES):
    import concourse.bacc as bacc
    import concourse.mybir as mybir
    from concourse import tile
    from concourse.library_config import mlp

    # NOTE: do NOT try to keep gather preps off the Tile DMASW lanes (e.g.
    # by shrinking tile_sem_assignment.DMAInst) to avoid the per-prep
    # InstIncSwdgeSem pads (~1.4us Pool each): the pads feed the SWDGE
    # shadow-semaphore/ring bookkeeping and removing them hangs the device.

    f16 = mybir.dt.float16
    nc = bacc.Bacc("TRN2", target_bir_lowering=False, debug=False,
                   dynamic_dma_scratch_size=65536, num_swdge_queues=4)
    fm = nc.dram_tensor("fm", [F + 1, ELEM], f16, kind="ExternalInput")
    idxw = nc.dram_tensor("idxw", [ntiles, P, T // 16], mybir.dt.int16, kind="ExternalInput")
    baryt = nc.dram_tensor("baryt", [ntiles, P, G, 3, 2], f16, kind="ExternalInput")
    ident = nc.dram_tensor("ident", [P, P], f16, kind="ExternalInput")
    out = nc.dram_tensor("out", [P, ntiles * T], f16, kind="ExternalOutput")

    with tile.TileContext(nc) as tc:
        nc.gpsimd.load_library(mlp)
        # gen_mode=0 gathers: the DMA-completion sem is the Tile DMASW lane
        # sem attached directly (then_inc) so every auto-generated dep is
        # real -- no IncSwdgeSem pads (~1.4us Pool each), no triggers, no
        # manual gating. Inline completion wait (~1-2us/call) is cheaper
        # than the prep-mode pad+trigger overhead it replaces.
        with (
            tc.tile_pool(name="const", bufs=1) as constp,
            tc.tile_pool(name="io", bufs=4) as iop,
            tc.tile_pool(name="work", bufs=4) as workp,
            tc.tile_pool(name="bounce", bufs=2) as bouncep,
            tc.tile_pool(name="ps", bufs=2, space="PSUM") as psump,
        ):
            id_sb = constp.tile([P, P], f16, tag="ident")
            nc.sync.dma_start(id_sb[:], ident[:])
            for t in range(ntiles):
                bary_sb = iop.tile([P, G, 3, 2], f16, tag="bary")
                idx_sb = iop.tile([P, T // 16], mybir.dt.int16, tag="idx")
                nc.sync.dma_start(bary_sb[:], baryt[t])
                nc.sync.dma_start(idx_sb[:], idxw[t])
                cw = CHUNK // 16
                attrs_chs = []
                with tc.high_priority(offset=400):
                    for ch in range(NCHUNK):
                        attrs_sb = iop.tile([P, GPC, ELEM], f16, tag=f"attrs{ch}")
                        attrs_chs.append(attrs_sb)
                        k = t * NCHUNK + ch
                        nc.gpsimd.dma_gather(
                            attrs_sb[:], fm[:],
                            idx_sb[:, ch * cw:(ch + 1) * cw],
                            CHUNK, CHUNK, ELEM,
                            queue_num=k % 4)
                summ = workp.tile([P, G, P], f16, tag="summ")
                for ch in range(NCHUNK):
                    k = t * NCHUNK + ch
                    attrs_sb = attrs_chs[ch]
                    prod = workp.tile([P, GPC, ELEM], f16, tag=f"prod{ch}")
                    a5 = attrs_sb[:].rearrange(
                        "p g (h v l) -> p g h v l", h=64, v=3, l=2)
                    b5 = bary_sb[:, ch * GPC:(ch + 1) * GPC, None, :, :] \
                        .to_broadcast((P, GPC, 64, 3, 2))
                    p5 = prod[:].rearrange(
                        "p g (h v l) -> p g h v l", h=64, v=3, l=2)
                    nc.vector.tensor_mul(p5, a5, b5)
                    s4 = summ[:, ch * GPC:(ch + 1) * GPC, :].rearrange(
                        "p g (h l) -> p g h l", h=64, l=2)
                    nc.vector.tensor_add(s4, p5[:, :, :, 0, :], p5[:, :, :, 1, :])
                    nc.vector.tensor_add(s4, s4, p5[:, :, :, 2, :])
                ps = psump.tile([P, T], f16, tag="ps")
                for g in range(G):
                    nc.tensor.matmul(
                        ps[:, g * P:(g + 1) * P], summ[:, g, :], id_sb[:],
                        is_transpose=True, start=True, stop=True,
                    )
                bounce = bouncep.tile([P, T], f16, tag="bounce")
                nc.scalar.copy(bounce[:], ps[:])
                nc.sync.dma_start(out[:, t * T:(t + 1) * T], bounce[:])
    nc.compile()
    return nc


def _get_nc():
    if "nc" not in _CACHE:
        _CACHE["nc"] = _build_nc()
    return _CACHE["nc"]


def _prep_in_maps(pix_to_face, bary_coords, face_memory):
    N, H, W, K = pix_to_face.shape          # 4, 512, 512, 1
    assert (N, H, W, K) == (4, 512, 512, 1)
    fm = np.asarray(face_memory, dtype=np.float32)           # [F, 3, 128]
    # interleave: fm16[f, h, v, l] = fm[f, v, 2h+l]  (channel c = 2h+l)
    fm16 = np.ascontiguousarray(
        fm.reshape(F, 3, 64, 2).transpose(0, 2, 1, 3)).astype(np.float16)
    fm16 = fm16.reshape(F, ELEM)
    fm_pad = np.concatenate([fm16, np.zeros((1, ELEM), np.float16)], axis=0)
    ident = np.eye(P, dtype=np.float16)

    idx_all = np.asarray(pix_to_face).reshape(N, H, W)
    bary_all = np.asarray(bary_coords, dtype=np.float32).reshape(N, H, W, 3)

    in_maps = []
    for c in range(N_CORES):
        n, hh = c // 2, (c % 2) * 256
        idx = idx_all[n, hh:hh + 256].reshape(-1)                 # [131072]
        bary = bary_all[n, hh:hh + 256].reshape(-1, 3)            # [131072, 3]
        idx16 = np.where(idx < 0, F, idx).astype(np.int16)
        # per chunk: wrap 16-way ([16, CHUNK/16]), replicate to 128 partitions;
        # chunks laid side by side along the free dim -> [nt, 128, T/16]
        idxw = np.ascontiguousarray(
            idx16.reshape(NTILES, NCHUNK, CHUNK // 16, 16).transpose(0, 1, 3, 2))
        idxw = np.tile(idxw, (1, 1, 8, 1))                  # [nt, nc, 128, CH/16]
        idxw = np.ascontiguousarray(
            idxw.transpose(0, 2, 1, 3).reshape(NTILES, P, T // 16))
        # bary2[t, p, g, v, l] = bary[pixel=(t,g,p), v] duplicated over l
        baryt = bary.reshape(NTILES, G, P, 3).transpose(0, 2, 1, 3)  # [nt,128,G,3]
        baryt2 = np.ascontiguousarray(
            np.repeat(baryt[..., None], 2, axis=-1)).astype(np.float16)
        in_maps.append({"fm": fm_pad, "idxw": idxw, "baryt": baryt2, "ident": ident})
    return in_maps


def _assemble(results):
    out_full = np.empty((4, 128, 512, 512), dtype=np.float32)
    for c in range(N_CORES):
        n, hh = c // 2, (c % 2) * 256
        out_full[n, :, hh:hh + 256, :] = (
            results[c]["out"].astype(np.float32).reshape(128, 256, 512))
    return out_full


def run(in_maps, trace=False, trace_kwargs=None):
    from concourse.bass_utils import run_bass_kernel_spmd

    nc = _get_nc()
    kw = {}
    if trace:
        kw = dict(trace=True, trace_kwargs=trace_kwargs or {})
    return run_bass_kernel_spmd(nc, in_maps, list(range(N_CORES)), **kw)


def kernel(pix_to_face, bary_coords, face_memory):
    in_maps = _prep_in_maps(pix_to_face, bary_coords, face_memory)
    res = run(in_maps)
    return _assemble(res.results)


# revision 58
# speedup vs baseline: 1.0199x; 1.0199x over previous
"""Trainium2 Bass kernel for MeshInterpolate (interpolate_face_attributes).

Problem (hardcoded shapes):
  pix_to_face [4, 512, 512, 1] int64 (-1 = background), values in [-1, 10000)
  bary_coords [4, 512, 512, 1, 3] f32
  face_memory [10000, 3, 128] f32
  output      [4, 128, 512, 512] f32 (NCHW)

Sharding: data-parallel over (N, H/2): 8 cores, core c handles image c//2,
rows 256*(c%2) .. +256  -> 131072 pixels per core. face_memory replicated.

v2 design (all fp16 on device; host casts the output back to f32):
  - face_memory rows pre-interleaved on host to [f, h(64), v(3), l(2)] fp16
    (channel c = 2h+l) so the bary product runs in DVE 2x mode with a
    pair-duplicated bary operand (inner dim stride-1 size-2 on every AP).
  - dma_gather(prepare_only=True) + trigger_dma: Pool only pays descriptor
    generation (~2us/1024 rows); SDMA drains overlap the next prep.
    Background pixels index a zero row appended at fm[10000] -> no masking.
  - per tile (T = 2048 pixels = G=16 blocks of 128):
      prod[p,g,h,v,l] = attrs[p,g,h,v,l] * bary2[p,g,-,v,l]   (1 DVE op, 2x)
      s01 = prod[v=0] + prod[v=1]; sum = s01 + prod[v=2]      (2 DVE ops, 2x)
      PE is_transpose matmul per 128-px block: psum[c, p'] (fp16 1 cyc/row)
      ACT copies psum f32 -> fp16 bounce; HWDGE DMA to out[128, npix] fp16.
"""

import os

import numpy as np

# Safety: recover wedged NeuronCores from a previous crashed process. Must be
# set before the first jax/NRT backend init in this process.
os.environ.setdefault("NEURON_RT_RESET_CORES", "1")

P = 128
ELEM = 384            # one face row: 3*128 fp16 = 768B
G = 16                # 128-pixel blocks per tile
T = G * P             # 2048 pixels per tile
CHUNK = 1024          # pixels per dma_gather call; each chunk gathers into
                      # its OWN tile (Tile misses consumer waits when two
                      # preps write disjoint slices of one tile). 2048 hangs
                      # the gather ucode on HW - do not raise.
NCHUNK = T // CHUNK   # gather chunks per tile
GPC = CHUNK // P      # g-blocks per chunk
NTILES = 64           # per-core tiles: 131072 pixels
F = 10000
N_CORES = 8
NPIX_CORE = NTILES * T

_CACHE = {}


def _build_nc(ntiles=NTILES):
    import concourse.bacc as bacc
    import concourse.mybir as mybir
    from concourse import tile
    from concourse.library_config import mlp

    # NOTE: do NOT try to keep gather preps off the Tile DMASW lanes (e.g.
    # by shrinking tile_sem_assignment.DMAInst) to avoid the per-prep
    # InstIncSwdgeSem pads (~1.4us Pool each): the pads feed the SWDGE
    # shadow-semaphore/ring bookkeeping and removing them hangs the device.

    f16 = mybir.dt.float16
    nc = bacc.Bacc("TRN2", target_bir_lowering=False, debug=False,
                   dynamic_dma_scratch_size=65536, num_swdge_queues=4)
    fm = nc.dram_tensor("fm", [F + 1, ELEM], f16, kind="ExternalInput")
    idxw = nc.dram_tensor("idxw", [ntiles, P, T // 16], mybir.dt.int16, kind="ExternalInput")
    baryt = nc.dram_tensor("baryt", [ntiles, P, G, 3, 2], f16, kind="ExternalInput")
    ident = nc.dram_tensor("ident", [P, P], f16, kind="ExternalInput")
    out = nc.dram_tensor("out", [P, ntiles * T], f16, kind="ExternalOutput")

    with tile.TileContext(nc) as tc:
        nc.gpsimd.load_library(mlp)
        # gen_mode=0 gathers: the DMA-completion sem is the Tile DMASW lane
        # sem attached directly (then_inc) so every auto-generated dep is
        # real -- no IncSwdgeSem pads (~1.4us Pool each), no triggers, no
        # manual gating. Inline completion wait (~1-2us/call) is cheaper
        # than the prep-mode pad+trigger overhead it replaces.
        with (
            tc.tile_pool(name="const", bufs=1) as constp,
            tc.tile_pool(name="io", bufs=4) as iop,
            tc.tile_pool(name="work", bufs=3) as workp,
            tc.tile_pool(name="bounce", bufs=3) as bouncep,
            tc.tile_pool(name="ps", bufs=2, space="PSUM") as psump,
        ):
            id_sb = constp.tile([P, P], f16, tag="ident")
            nc.sync.dma_start(id_sb[:], ident[:])
            for t in range(ntiles):
                bary_sb = iop.tile([P, G, 3, 2], f16, tag="bary")
                idx_sb = iop.tile([P, T // 16], mybir.dt.int16, tag="idx")
                nc.sync.dma_start(bary_sb[:], baryt[t])
                nc.sync.dma_start(idx_sb[:], idxw[t])
                cw = CHUNK // 16
                attrs_chs = []
                with tc.high_priority(offset=400):
                    for ch in range(NCHUNK):
                        attrs_sb = iop.tile([P, GPC, ELEM], f16, tag=f"attrs{ch}")
                        attrs_chs.append(attrs_sb)
                        k = t * NCHUNK + ch
                        nc.gpsimd.dma_gather(
                            attrs_sb[:], fm[:],
                            idx_sb[:, ch * cw:(ch + 1) * cw],
                            CHUNK, CHUNK, ELEM,
                            queue_num=k % 4)
                summ = workp.tile([P, G, P], f16, tag="summ")
                for ch in range(NCHUNK):
                    k = t * NCHUNK + ch
                    attrs_sb = attrs_chs[ch]
                    prod = workp.tile([P, GPC, ELEM], f16, tag=f"prod{ch}")
                    a5 = attrs_sb[:].rearrange(
                        "p g (h v l) -> p g h v l", h=64, v=3, l=2)
                    b5 = bary_sb[:, ch * GPC:(ch + 1) * GPC, None, :, :] \
                        .to_broadcast((P, GPC, 64, 3, 2))
                    p5 = prod[:].rearrange(
                        "p g (h v l) -> p g h v l", h=64, v=3, l=2)
                    nc.vector.tensor_mul(p5, a5, b5)
                    s4 = summ[:, ch * GPC:(ch + 1) * GPC, :].rearrange(
                        "p g (h l) -> p g h l", h=64, l=2)
                    nc.vector.tensor_add(s4, p5[:, :, :, 0, :], p5[:, :, :, 1, :])
                    nc.vector.tensor_add(s4, s4, p5[:, :, :, 2, :])
                ps = psump.tile([P, T], f16, tag="ps")
                for g in range(G):
                    nc.tensor.matmul(
                        ps[:, g * P:(g + 1) * P], summ[:, g, :], id_sb[:],
                        is_transpose=True, start=True, stop=True,
                    )
                bounce = bouncep.tile([P, T], f16, tag="bounce")
                nc.scalar.copy(bounce[:], ps[:])
                nc.sync.dma_start(out[:, t * T:(t + 1) * T], bounce[:])
    nc.compile()
    return nc


def _get_nc():
    if "nc" not in _CACHE:
        _CACHE["nc"] = _build_nc()
    return _CACHE["nc"]


def _prep_in_maps(pix_to_face, bary_coords, face_memory):
    N, H, W, K = pix_to_face.shape          # 4, 512, 512, 1
    assert (N, H, W, K) == (4, 512, 512, 1)
    fm = np.asarray(face_memory, dtype=np.float32)           # [F, 3, 128]
    # interleave: fm16[f, h, v, l] = fm[f, v, 2h+l]  (channel c = 2h+l)
    fm16 = np.ascontiguousarray(
        fm.reshape(F, 3, 64, 2).transpose(0, 2, 1, 3)).astype(np.float16)
    fm16 = fm16.reshape(F, ELEM)
    fm_pad = np.concatenate([fm16, np.zeros((1, ELEM), np.float16)], axis=0)
    ident = np.eye(P, dtype=np.float16)

    idx_all = np.asarray(pix_to_face).reshape(N, H, W)
    bary_all = np.asarray(bary_coords, dtype=np.float32).reshape(N, H, W, 3)

    in_maps = []
    for c in range(N_CORES):
        n, hh = c // 2, (c % 2) * 256
        idx = idx_all[n, hh:hh + 256].reshape(-1)                 # [131072]
        bary = bary_all[n, hh:hh + 256].reshape(-1, 3)            # [131072, 3]
        idx16 = np.where(idx < 0, F, idx).astype(np.int16)
        # per chunk: wrap 16-way ([16, CHUNK/16]), replicate to 128 partitions;
        # chunks laid side by side along the free dim -> [nt, 128, T/16]
        idxw = np.ascontiguousarray(
            idx16.reshape(NTILES, NCHUNK, CHUNK // 16, 16).transpose(0, 1, 3, 2))
        idxw = np.tile(idxw, (1, 1, 8, 1))                  # [nt, nc, 128, CH/16]
        idxw = np.ascontiguousarray(
            idxw.transpose(0, 2, 1, 3).reshape(NTILES, P, T // 16))
        # bary2[t, p, g, v, l] = bary[pixel=(t,g,p), v] duplicated over l
        baryt = bary.reshape(NTILES, G, P, 3).transpose(0, 2, 1, 3)  # [nt,128,G,3]
        baryt2 = np.ascontiguousarray(
            np.repeat(baryt[..., None], 2, axis=-1)).astype(np.float16)
        in_maps.append({"fm": fm_pad, "idxw": idxw, "baryt": baryt2, "ident": ident})
    return in_maps


def _assemble(results):
    out_full = np.empty((4, 128, 512, 512), dtype=np.float32)
    for c in range(N_CORES):
        n, hh = c // 2, (c % 2) * 256
        out_full[n, :, hh:hh + 256, :] = (
            results[c]["out"].astype(np.float32).reshape(128, 256, 512))
    return out_full


def run(in_maps, trace=False, trace_kwargs=None):
    from concourse.bass_utils import run_bass_kernel_spmd

    nc = _get_nc()
    kw = {}
    if trace:
        kw = dict(trace=True, trace_kwargs=trace_kwargs or {})
    return run_bass_kernel_spmd(nc, in_maps, list(range(N_CORES)), **kw)


def kernel(pix_to_face, bary_coords, face_memory):
    in_maps = _prep_in_maps(pix_to_face, bary_coords, face_memory)
    res = run(in_maps)
    return _assemble(res.results)
